# revision 1
# baseline (speedup 1.0000x reference)
"""Masked attention kernel for Trainium2, 8 NeuronCores.

B=2, H=8, S=4096, D=64 attention, shared bool mask (True = -1e9 fill
before /sqrt(D)), softmax over keys, @ V.

Sharding: 2 head-groups x 4 query-slabs. Each core gets 8 heads and a
1024-query slab. The mask slab (4096 x 1024 fp16 keep matrix, 8MB) is
read ONCE per core and reused by all 8 heads -- 4x less mask traffic
than pure head-parallel sharding.

Per core, per 512-query strip, per head (scores transposed: keys on
partitions):
  S^T[k,q] = K @ Q^T   rowpacked: two 64-contraction matmuls run
                       concurrently in PE row groups (0,0)/(64,0),
                       ktile 2j in rows 0-63, ktile 2j+1 in rows 64-127
  P^T = exp(S^T/8)     ACT, PSUM->SBUF fp16, one instr per ktile pair
  P^T *= keepT         DVE fp16 (4x mode)
  O^T[65,q] += [V|1]^T @ P^T   PE, contraction 128, ones col = denom
  epilogue: PE transpose, DVE reciprocal+scale, DMA out
"""

import os
import sys

import numpy as np

for _p in ("/opt/trn_rl_repo",):
    if os.path.isdir(_p) and _p not in sys.path:
        sys.path.insert(0, _p)

B, H, S, D = 2, 8, 4096, 64
N_CORES = 8
HG = 2          # head groups
QG = N_CORES // HG  # query-slab groups
HPC = (B * H) // HG  # heads per core = 8
QSLAB = S // QG      # queries per core = 1024
QSTRIP = 512
KSUP = 3  # ktiles per exp/mask/PV chunk


def build_program(
    reps=1,
    with_chain=False,
    rowpack=True,
    ksup=KSUP,
    runtime_reps=False,
    colpack=False,
    notranspose=False,
    warmup=40,
):
    import concourse.bacc as bacc
    import concourse.mybir as mybir
    import concourse.tile as tile
    from concourse.masks import make_identity

    f16 = mybir.dt.float16
    f32 = mybir.dt.float32
    Exp = mybir.ActivationFunctionType.Exp

    s = S
    hpc = HPC
    qstrip = QSTRIP
    ktiles = s // 128          # 32
    npair = ktiles // 2        # 16
    nstrip = QSLAB // qstrip   # 2
    nsub = qstrip // 128       # 4
    chunk_sz = [ksup] * (ktiles // ksup)
    if ktiles % ksup:
        chunk_sz.append(ktiles % ksup)
    chunk_start = [sum(chunk_sz[:i]) for i in range(len(chunk_sz))]

    nc = bacc.Bacc(None, target_bir_lowering=False)
    if with_chain:
        nc.dram_tensor("CHAIN", [1], f32, kind="ExternalInput")
    RCOUNT = None
    if runtime_reps:
        RCOUNT = nc.dram_tensor(
            "RCOUNT", [1, 1], mybir.dt.int32, kind="ExternalInput"
        )
    # K rowpack-interleaved: [0:64,h,p*128+i] = K^T[:, ktile 2p key i],
    #                        [64:128,h,p*128+i] = K^T[:, ktile 2p+1 key i]
    KTP = nc.dram_tensor("KTP", [128, hpc, s // 2], f16, kind="ExternalInput")
    # Q^T duplicated on both partition halves
    QT2 = nc.dram_tensor("QT2", [128, hpc, QSLAB], f16, kind="ExternalInput")
    VP = nc.dram_tensor("VP", [128, hpc, ktiles, 65], f16, kind="ExternalInput")
    # keep^T slab for this core's queries: [key, q]
    KEEP = nc.dram_tensor("KEEP", [s, QSLAB], f16, kind="ExternalInput")
    if colpack:
        # unnormalized O^T (d-major) + 4 denominator partials per head;
        # host merges partials, divides, transposes
        O = nc.dram_tensor("OT", [hpc, 64, QSLAB], f32, kind="ExternalOutput")
        DEN = nc.dram_tensor(
            "DEN", [hpc, 4, QSLAB], f32, kind="ExternalOutput"
        )
    elif notranspose:
        O = nc.dram_tensor("OT", [hpc, 65, QSLAB], f32, kind="ExternalOutput")
    else:
        O = nc.dram_tensor("O", [hpc, QSLAB, 64], f32, kind="ExternalOutput")

    with tile.TileContext(nc) as tc:
        with (
            tc.tile_pool(name="persist", bufs=1) as persist,
            tc.tile_pool(name="keepp", bufs=2) as keepp,
            tc.tile_pool(name="pp", bufs=4) as pp,
            tc.tile_pool(name="sp", bufs=2, space="PSUM") as sp,
            tc.tile_pool(
                name="op", bufs=(2 if ksup <= 2 else 1), space="PSUM"
            ) as op,
            tc.tile_pool(name="tp", bufs=1, space="PSUM") as tp,
            tc.tile_pool(name="ep", bufs=2) as ep,
            tc.tile_pool(name="dp", bufs=1, space="PSUM") as dp,
        ):
            kt = persist.tile([128, hpc, s // 2], f16)
            qt = persist.tile([128, hpc, QSLAB], f16)
            vt = persist.tile([128, hpc, ktiles, 65], f16)
            kstrips = [
                persist.tile([128, ktiles, qstrip], f16, name=f"kstrip{i}")
                for i in range(nstrip)
            ]

            def emit_kstrip_chunks(t, q0, chunk_sizes, c0=0):
                for csz in chunk_sizes:
                    nc.sync.dma_start(
                        out=t[:, c0 : c0 + csz, :],
                        in_=KEEP[
                            c0 * 128 : (c0 + csz) * 128, q0 : q0 + qstrip
                        ].rearrange("(j p) q -> p j q", p=128),
                    )
                    c0 += csz

            # DMA order: strip-0 critical path first (first heads' K/Q,
            # first mask chunks, first V), then the rest.
            # first slices only, so head-0 chunk-0 compute starts asap
            nc.sync.dma_start(out=kt[:, 0, 0:512], in_=KTP[:, 0, 0:512])
            nc.sync.dma_start(out=qt[:, 0, 0:qstrip], in_=QT2[:, 0, 0:qstrip])
            emit_kstrip_chunks(kstrips[0], 0, [4])
            nc.sync.dma_start(out=vt[:, 0, 0:4], in_=VP[:, 0, 0:4])
            if warmup:
                # dummy matmuls fill the initial DMA wait so the PE HAM
                # un-throttles before the first real matmul; reuse the
                # ott pool slot so no extra PSUM bank is needed
                wq = persist.tile([64, 512], f16, name="warmq")
                nc.vector.memset(wq, 0.0)
                wps = tp.tile([128, nsub, 65], f32, name="ott")
                for _w in range(warmup):
                    nc.tensor.matmul(
                        wps[:, 0, :], lhsT=wq[:, 0:128], rhs=wq[:, 0:65],
                        start=True, stop=True,
                    )
            nc.sync.dma_start(out=kt[:, 0, 512:], in_=KTP[:, 0, 512:])
            nc.sync.dma_start(out=qt[:, 0, qstrip:], in_=QT2[:, 0, qstrip:])
            nc.sync.dma_start(out=vt[:, 0, 4:], in_=VP[:, 0, 4:])
            for h in range(1, hpc):
                nc.sync.dma_start(out=kt[:, h, :], in_=KTP[:, h, :])
                nc.sync.dma_start(out=qt[:, h, :], in_=QT2[:, h, :])
                nc.sync.dma_start(out=vt[:, h], in_=VP[:, h])
            emit_kstrip_chunks(kstrips[0], 0, [7] * 4, c0=4)
            for si in range(1, nstrip):
                emit_kstrip_chunks(kstrips[si], si * qstrip, [8] * 4)
            ident = persist.tile([128, 128], f32)
            make_identity(nc, ident)
            ones1 = None
            if colpack:
                ones1 = persist.tile([128, 32], f16, name="ones1")
                nc.vector.memset(ones1, 1.0)

            pending_epilogue = []

            def emit_epilogue_nt(osb, h, q0):
                # normalize in O^T orientation: rec row -> broadcast -> mul
                rec = ep.tile([1, qstrip], f32, name="recnt")
                nc.vector.reciprocal(rec, osb[64:65, :])
                pb = ep.tile([64, qstrip], f32, name="pbnt")
                nc.gpsimd.partition_broadcast(pb, rec)
                of = ep.tile([64, qstrip], f32, name="ofnt")
                nc.vector.tensor_mul(of, osb[0:64, :], pb)
                nc.sync.dma_start(
                    out=O[h, 0:64, q0 : q0 + qstrip], in_=of
                )

            def emit_epilogue(osb, h, q0):
                if notranspose:
                    return emit_epilogue_nt(osb, h, q0)
                ott = tp.tile([128, nsub, 65], f32, name="ott")
                for t in range(nsub):
                    nc.tensor.transpose(
                        ott[:, t, :],
                        osb[:, t * 128 : (t + 1) * 128],
                        ident[:65, :65],
                    )
                rec = ep.tile([128, nsub], f32, name="rec")
                nc.vector.reciprocal(rec, ott[:, :, 64])
                of = ep.tile([128, nsub, 64], f32, name="of")
                for t in range(nsub):
                    nc.vector.tensor_scalar_mul(
                        of[:, t, :], ott[:, t, :64], rec[:, t : t + 1]
                    )
                nc.sync.dma_start(
                    out=O[h, q0 : q0 + qstrip, :].rearrange(
                        "(t p) d -> p t d", p=128
                    ),
                    in_=of,
                )

            def emit_epilogue_colpack(pvacc, dacc, h, q0):
                # O^T halves add: PSUM + SBUF-copy, then DMA d-major
                h2sb = ep.tile([64, qstrip], f32, name="h2sb")
                nc.vector.tensor_copy(h2sb, pvacc[64:128, :])
                osum = ep.tile([64, qstrip], f32, name="osum")
                nc.vector.tensor_add(osum, pvacc[0:64, :], h2sb)
                nc.sync.dma_start(
                    out=O[h, :, q0 : q0 + qstrip], in_=osum
                )
                dsb = ep.tile([128, qstrip], f32, name="dsb")
                nc.vector.tensor_copy(dsb, dacc)
                for g in range(4):
                    nc.sync.dma_start(
                        out=DEN[h, g : g + 1, q0 : q0 + qstrip],
                        in_=dsb[32 * g : 32 * g + 1, :],
                    )

            import contextlib

            if runtime_reps:
                tmp = nc.alloc_registers("rcount_tmp", mybir.ALL_ENGINES)
                nc.regs_load(tmp, RCOUNT[0:1, 0:1])
                rv = nc.snap(tmp, donate=True, min_val=1, max_val=1 << 20)
                rep_ctx = tc.For_i(0, rv)
            elif reps > 1:
                rep_ctx = tc.For_i(0, reps)
            else:
                rep_ctx = contextlib.nullcontext()
            with rep_ctx:
                for si in range(nstrip):
                    q0 = si * qstrip
                    kstrip = kstrips[si]
                    for h in range(hpc):
                        if colpack:
                            pvacc = op.tile([128, qstrip], f32, name="pvacc")
                            dacc = dp.tile([128, qstrip], f32, name="dacc")
                        else:
                            oacc = op.tile([65, qstrip], f32, name="oacc")
                        for ck, (c0, csz) in enumerate(
                            zip(chunk_start, chunk_sz)
                        ):
                            st = sp.tile([128, ksup, qstrip], f32)
                            for t in range(csz):
                                ktile = c0 + t
                                pr = ktile // 2
                                base = 64 * (ktile % 2)
                                tp_arg = (
                                    {"tile_position": (base, 0)}
                                    if rowpack
                                    else {}
                                )
                                nc.tensor.matmul(
                                    st[:, t, :],
                                    lhsT=kt[
                                        base : base + 64,
                                        h,
                                        pr * 128 : (pr + 1) * 128,
                                    ],
                                    rhs=qt[base : base + 64, h, q0 : q0 + qstrip],
                                    start=True,
                                    stop=True,
                                    **tp_arg,
                                )
                            pt = pp.tile([128, ksup, qstrip], f16)
                            nc.scalar.activation(
                                pt[:, :csz, :], st[:, :csz, :], Exp, scale=0.125
                            )
                            nc.vector.tensor_mul(
                                pt[:, :csz, :],
                                pt[:, :csz, :],
                                kstrip[:, c0 : c0 + csz, :],
                            )
                            if colpack:
                                for t in range(csz):
                                    ktile = c0 + t
                                    half = ktile % 2
                                    nc.tensor.matmul(
                                        pvacc[64 * half : 64 * half + 64, :],
                                        lhsT=vt[:, h, ktile, 0:64],
                                        rhs=pt[:, t, :],
                                        start=(ktile < 2),
                                        stop=(ktile >= ktiles - 2),
                                        tile_position=(0, 64 * half),
                                        skip_group_check=True,
                                    )
                                for t in range(csz):
                                    ktile = c0 + t
                                    g = ktile % 4
                                    nc.tensor.matmul(
                                        dacc[32 * g : 32 * g + 32, :],
                                        lhsT=ones1,
                                        rhs=pt[:, t, :],
                                        start=(ktile < 4),
                                        stop=(ktile >= ktiles - 4),
                                        tile_position=(0, 32 * g),
                                        skip_group_check=True,
                                    )
                            else:
                                for t in range(csz):
                                    ktile = c0 + t
                                    nc.tensor.matmul(
                                        oacc,
                                        lhsT=vt[:, h, ktile, :],
                                        rhs=pt[:, t, :],
                                        start=(ktile == 0),
                                        stop=(ktile == ktiles - 1),
                                    )
                            if (
                                ck == 3
                                and not colpack
                                and pending_epilogue
                            ):
                                emit_epilogue(*pending_epilogue.pop())
                        if colpack:
                            emit_epilogue_colpack(pvacc, dacc, h, q0)
                        else:
                            osb = ep.tile(
                                [65, qstrip], f32, name="osb", bufs=3
                            )
                            nc.vector.tensor_copy(osb, oacc)
                            pending_epilogue.append((osb, h, q0))
                while pending_epilogue:
                    emit_epilogue(*pending_epilogue.pop())
    nc.compile()
    return nc


def prep_inputs(Q, K, V, mask):
    """Host-side marshalling: fp16 casts, transposes, per-core shards."""
    s, hpc, qslab = S, HPC, QSLAB
    nheads = B * H
    Qr = np.asarray(Q, dtype=np.float32).reshape(nheads, s, D)
    Kr = np.asarray(K, dtype=np.float32).reshape(nheads, s, D)
    Vr = np.asarray(V, dtype=np.float32).reshape(nheads, s, D)
    keepT = np.ascontiguousarray(
        (~np.asarray(mask).reshape(s, s)).T.astype(np.float16)
    )
    in_maps = []
    for c in range(N_CORES):
        hg, qg = c // QG, c % QG
        hsl = slice(hg * hpc, (hg + 1) * hpc)
        qsl = slice(qg * qslab, (qg + 1) * qslab)
        # K^T [D, hpc, S] -> rowpack interleave [128, hpc, S/2]
        ktc = Kr[hsl].transpose(2, 0, 1).astype(np.float16)  # [64, hpc, S]
        ktc = ktc.reshape(64, hpc, s // 256, 2, 128)
        ktp = np.concatenate([ktc[:, :, :, 0, :], ktc[:, :, :, 1, :]], axis=0)
        ktp = np.ascontiguousarray(ktp.reshape(128, hpc, s // 2))
        # Q^T slab duplicated on partition halves
        qtc = Qr[hsl, qsl].transpose(2, 0, 1).astype(np.float16)  # [64,hpc,qslab]
        qt2 = np.ascontiguousarray(np.concatenate([qtc, qtc], axis=0))
        # V + ones col: [128, hpc, ktiles, 65]
        vpc = np.ones((128, hpc, s // 128, 65), dtype=np.float16)
        vpc[:, :, :, :64] = (
            Vr[hsl].reshape(hpc, s // 128, 128, D).transpose(2, 0, 1, 3)
        ).astype(np.float16)
        in_maps.append(
            {
                "KTP": ktp,
                "QT2": qt2,
                "VP": vpc,
                "KEEP": np.ascontiguousarray(keepT[:, qsl]),
            }
        )
    return in_maps


COLPACK = False
NOTRANSPOSE = False


def gather_outputs(results, colpack=COLPACK, notranspose=NOTRANSPOSE):
    full = np.empty((B * H, S, D), dtype=np.float32)
    for c, r in enumerate(results):
        hg, qg = c // QG, c % QG
        hsl = slice(hg * HPC, (hg + 1) * HPC)
        qsl = slice(qg * QSLAB, (qg + 1) * QSLAB)
        if colpack:
            den = r["DEN"].sum(axis=1)  # [hpc, qslab]
            o = r["OT"] / den[:, None, :]  # [hpc, 64, qslab]
            full[hsl, qsl, :] = o.transpose(0, 2, 1)
        elif notranspose:
            full[hsl, qsl, :] = r["OT"][:, 0:64, :].transpose(0, 2, 1)
        else:
            full[hsl, qsl, :] = r["O"]
    return np.ascontiguousarray(full.reshape(B, H, S, D))


_CACHE = {}


def get_program():
    if "nc" not in _CACHE:
        _CACHE["nc"] = build_program(colpack=COLPACK, notranspose=NOTRANSPOSE)
    return _CACHE["nc"]


def kernel(Q, K, V, mask):
    from concourse.bass_utils import run_bass_kernel_spmd

    nc = get_program()
    in_maps = prep_inputs(Q, K, V, mask)
    res = run_bass_kernel_spmd(nc, in_maps, core_ids=list(range(N_CORES)))
    return gather_outputs(res.results)


if __name__ == "__main__":
    import jax

    sys.path.insert(0, os.path.dirname(os.path.abspath(__file__)))
    import reference

    with jax.default_device(jax.devices("cpu")[0]):
        inputs = {k: np.asarray(v) for k, v in reference.setup_inputs().items()}
        expected = np.asarray(reference.reference(**inputs))
    actual = kernel(**inputs)
    err = np.abs(actual - expected).max() / np.abs(expected).max()
    print("Relative error:", err)

